# revision 22
# baseline (speedup 1.0000x reference)
"""Trainium2 Bass kernel for nn_Attention_36601711297049.

Self-attention (4 heads, dim_head 32) over N=4096 tokens, batch 2:
  qkv = w_qkv @ x ; sim = scale * q^T k ; attn = softmax(sim) ;
  out = attn @ v ; y = w_out @ out + b_out

Sharding: 8 cores = 2 batches x 4 query-chunks (1024 queries each).
Each core computes k, v for the full batch plus q for its own chunk, runs
flash-style attention in S^T layout ([keys, queries] so AV needs no
transposes), and applies the output projection locally. No collectives.
Softmax skips max-subtraction (logits are ~N(0,1), safely in fp32 range).

Host-side trick: each core's x arrives with key pieces permuted so piece 0
IS the core's query chunk (softmax/AV are key-order invariant) - the q gemm
starts as soon as the first DMA lands.

ScalarE exp is the floor (~2.4us per iteration); the PE work (S^T 4-tile
blocks, AV col-pairs lagging one step, batched qkv gemms) hides under it.
PSUM pool tiles rotate as separate memrefs - a single shared tile would
serialize on the framework's coarse write-after-read tracking.
"""
import sys

for p in ("/opt/trn_rl_repo", "/root/.axon_site/_ro/trn_rl_repo"):
    if p not in sys.path:
        sys.path.insert(0, p)

import numpy as np
from contextlib import ExitStack

import concourse.bass as bass
from concourse import bacc
import concourse.tile as tile
from concourse import mybir
from concourse.bass_utils import run_bass_kernel_spmd

F32 = mybir.dt.float32
BF16 = mybir.dt.bfloat16
AF = mybir.ActivationFunctionType

# ---- custom DVE exp: exp(c0*x) = (T3(c0*x))^64, two instructions ----
# Seed: u = x*c0 ; p = (u + 1) + u^2*(1/2 + u/6). Squarer: p^64.
# Max rel err ~4e-4 for |c0*x| <= 7 (bf16 output quantization dominates).
from concourse import dve_ops as _DO
from concourse.dve_spec import (Spec as _Spec, Src0 as _S0, Src1 as _S1,
                                C0 as _C0, C1 as _C1, C2 as _C2,
                                lower as _lower, _has_src1, sq as _sq)
from concourse.dve_uop import DveOpSpec as _DveOpSpec
from concourse.dve_ops import DveOp as _DveOp


def _ref_exp_seed(in0, in1, c0, c1, c2):
    u = (in0.astype(np.float32) * np.float32(c0)).astype(np.float32)
    return ((u + in1) + (u * u) * (np.float32(c1) + u * np.float32(c2))).astype(
        np.float32)


def _ref_exp_sq6(in0, in1, c0, c1, c2):
    q = in0.astype(np.float32)
    for _ in range(6):
        q = (q * q).astype(np.float32)
    return q


def _register_dve_exp():
    if "ANT_EXP64_SEED" in _DO._SUB_OPCODE_FOR_NAME:
        byname = {op.name: op for op in _DO.OPS}
        return byname["ANT_EXP64_SEED"], byname["ANT_EXP64_SQ6"]
    _u = _S0 * _C0
    spec1 = _Spec(body=(_u + _S1) + _sq(_u) * (_C1 + _u * _C2),
                  reference=_ref_exp_seed)
    q1 = _sq(_S0)
    for _ in range(4):
        q1 = _sq(q1)
    spec2 = _Spec(body=_sq(q1), reference=_ref_exp_sq6)
    ops = []
    for name, spec in (("ANT_EXP64_SEED", spec1), ("ANT_EXP64_SQ6", spec2)):
        row = max(_DO._SUB_OPCODE_FOR_NAME.values()) + 1
        _DO._SUB_OPCODE_FOR_NAME[name] = row
        shas = {}
        for ver in ("v3", "v4"):
            tmp = _DveOpSpec(name=name, opcode=row, uops=_lower(spec, ver=ver),
                             rd1_en=_has_src1(spec))
            shas[ver] = tmp.sha(ver)
        op = _DveOp(name, spec, subdim=False, uops_sha=shas)
        _DO.OPS.append(op)
        _DO.CUSTOM_DVE_SPECS[name] = spec
        ops.append(op)
    return ops[0], ops[1]


EXP_SEED_OP, EXP_SQ6_OP = _register_dve_exp()

HEADS = 4
DH = 32
C = 256          # channels
N = 4096         # h*w tokens per batch
QC = 1024        # queries per core
NK = N // 128    # 128-key tiles
SCALE = float(DH) ** -0.5


def build_nc():
    nc = bacc.Bacc("TRN2", target_bir_lowering=False)
    x = nc.dram_tensor("x", [C, N], BF16, kind="ExternalInput")
    wq = nc.dram_tensor("wq", [128, 2, 128], BF16, kind="ExternalInput")  # [p, cc, (h,d)]
    wk = nc.dram_tensor("wk", [128, 2, 128], BF16, kind="ExternalInput")
    wv = nc.dram_tensor("wv", [128, 2, 128], BF16, kind="ExternalInput")
    woA = nc.dram_tensor("woA", [128, C], BF16, kind="ExternalInput")  # w_out^T h0/h1 + bias row
    woB = nc.dram_tensor("woB", [128, C], BF16, kind="ExternalInput")  # w_out^T h2/h3, zero-padded
    out = nc.dram_tensor("out", [C, QC], F32, kind="ExternalOutput")

    with tile.TileContext(nc) as tc, ExitStack() as ctx:
        big = ctx.enter_context(tc.tile_pool(name="big", bufs=1))
        small = ctx.enter_context(tc.tile_pool(name="small", bufs=2))
        ptp = ctx.enter_context(tc.tile_pool(name="ptp", bufs=18))
        stp = ctx.enter_context(tc.tile_pool(name="stp", bufs=3, space="PSUM"))
        avp = ctx.enter_context(tc.tile_pool(name="avp", bufs=2, space="PSUM"))

        # warm the exp table set early (one tiny ACT forces the table load)
        dummy = small.tile([1, 8], F32, tag="dummy")
        nc.vector.memset(dummy[:], 0.0)
        nc.scalar.activation(dummy[:], dummy[:], AF.Exp)

        # ---- weights first (tiny, on sync), then x pieces ----
        wq_bf = big.tile([128, 2, 128], BF16, tag="wq_bf")
        wk_bf = big.tile([128, 2, 128], BF16, tag="wk_bf")
        wv_bf = big.tile([128, 2, 128], BF16, tag="wv_bf")
        for (dram, sbuf) in ((wq, wq_bf), (wk, wk_bf), (wv, wv_bf)):
            nc.sync.dma_start(sbuf[:], dram[:])
        woA_bf = big.tile([128, 256], BF16, tag="woA_bf")
        woB_bf = big.tile([128, 256], BF16, tag="woB_bf")
        nc.sync.dma_start(woA_bf[:], woA[:])
        nc.sync.dma_start(woB_bf[:], woB[:])
        ones_bf = big.tile([128, DH], BF16, tag="ones_bf")
        nc.vector.memset(ones_bf[:], 1.0)
        ones_f = big.tile([128, 1024], F32, tag="ones_f")
        nc.vector.memset(ones_f[:], 1.0)
        sdp = ctx.enter_context(tc.tile_pool(name="sdp", bufs=3))

        x_bf = big.tile([128, 2, N], BF16, tag="x_bf")
        dma_engines = (nc.sync, nc.gpsimd)

        # piece 0 = this core's query chunk; scalar queue helps only here
        nc.gpsimd.dma_start(x_bf[:, 0, 0:512], x[0:128, 0:512])
        nc.scalar.dma_start(x_bf[:, 0, 512:1024], x[0:128, 512:1024])
        nc.sync.dma_start(x_bf[:, 1, 0:1024], x[128:256, 0:1024])

        def dma_x_piece(piece, di):
            sl = slice(1024 * piece, 1024 * (piece + 1))
            for cc in range(2):
                dma_engines[(di + cc) % 2].dma_start(
                    x_bf[:, cc, sl], x[128 * cc:128 * (cc + 1), sl])

        for piece in range(1, 4):
            dma_x_piece(piece, 2 * piece)

        # ---- q = wq^T x[:, :, 0:QC] : [128 (h,d), QC] bf16 ----
        q_bf = big.tile([128, QC], BF16, tag="q_bf")
        for nch in range(QC // 512):
            ps = stp.tile([128, 1024], F32, tag="st", name=f"q_ps{nch}")
            for cc in range(2):
                nc.tensor.matmul(ps[:, :512], wq_bf[:, cc, :],
                                 x_bf[:, cc, 512 * nch:512 * (nch + 1)],
                                 start=(cc == 0), stop=(cc == 1),
                                 skip_group_check=True)
            nc.vector.tensor_copy(q_bf[:, 512 * nch:512 * (nch + 1)], ps[:, :512])

        # ---- k = wk^T x and vT = x^T wv, batched, trickled into the loop ----
        k_bf = big.tile([128, N], BF16, tag="k_bf")
        vT_bf = big.tile([128, NK, 4, 34], BF16, tag="vT_bf")
        for h in range(HEADS):
            nc.vector.memset(vT_bf[:, :, h, 32:33], 1.0)

        def emit_k_gemm(nch):  # 512 keys
            ps = stp.tile([128, 1024], F32, tag="st", name=f"k_ps{nch}")
            for cc in range(2):
                nc.tensor.matmul(ps[:, :512], wk_bf[:, cc, :],
                                 x_bf[:, cc, 512 * nch:512 * (nch + 1)],
                                 start=(cc == 0), stop=(cc == 1),
                                 skip_group_check=True)
            nc.vector.tensor_copy(k_bf[:, 512 * nch:512 * (nch + 1)], ps[:, :512])

        def emit_vT_batch(t):  # 4 key tiles (kt = 4t..4t+3) back-to-back
            ps = stp.tile([128, 1024], F32, tag="st", name=f"v_ps{t}")
            for j in range(4):
                kt = 4 * t + j
                for cc in range(2):
                    nc.tensor.matmul(ps[:, 128 * j:128 * (j + 1)],
                                     x_bf[:, cc, 128 * kt:128 * (kt + 1)],
                                     wv_bf[:, cc, :],
                                     start=(cc == 0), stop=(cc == 1),
                                     skip_group_check=True)
            for j in range(4):
                kt = 4 * t + j
                nc.vector.tensor_copy(
                    vT_bf[:, kt, :, 0:32],
                    ps[:, 128 * j:128 * (j + 1)].rearrange(
                        "p (h d) -> p h d", d=32))

        # ---- attention main loop ----
        avbs = {}
        pts_store = {}

        def emit_st_exp(qc, kt):
            """S^T 4-tile block + 2 exps for (qc, kt)."""
            qsl = slice(512 * qc, 512 * (qc + 1))
            st0 = stp.tile([128, 1024], F32, tag="st", name=f"st0_{qc}_{kt}")
            st1 = stp.tile([128, 1024], F32, tag="st", name=f"st1_{qc}_{kt}")
            sts = (st0, st0, st1, st1)
            for h in range(HEADS):
                nc.tensor.matmul(
                    sts[h][:, 512 * (h % 2):512 * (h % 2 + 1)],
                    k_bf[32 * h:32 * (h + 1), 128 * kt:128 * (kt + 1)],
                    q_bf[32 * h:32 * (h + 1), qsl],
                    start=True, stop=True, skip_group_check=True,
                    tile_position=(32 * h, 0))
            pt0 = ptp.tile([128, 1024], BF16, tag="pt", name=f"pt0_{qc}_{kt}")
            pt1 = ptp.tile([128, 1024], BF16, tag="pt", name=f"pt1_{qc}_{kt}")
            nc.scalar.activation(pt0[:], st0[:], AF.Exp, scale=SCALE)
            # offload st1's exp to the DVE where the PE isn't binding
            if (qc == 1 or kt >= 26) and kt % 2 == 0:
                sd = sdp.tile([128, 1024], F32, tag="sd", name=f"sd_{qc}_{kt}")
                nc.vector._custom_dve(EXP_SEED_OP, out=sd[:], in0=st1[:],
                                      in1=ones_f[:], s0=SCALE / 64.0,
                                      s1=0.5, imm2=1.0 / 6.0)
                nc.vector._custom_dve(EXP_SQ6_OP, out=pt1[:], in0=sd[:], s0=0.0)
            else:
                nc.scalar.activation(pt1[:], st1[:], AF.Exp, scale=SCALE)
            pts_store[(qc, kt)] = (pt0, pt1)

        def emit_av(qc, kt):
            if kt == 0:
                avbs[qc] = [avp.tile([128, 512], F32, tag="acc", name=f"av{qc}_{b}")
                            for b in range(2)]
            pt0, pt1 = pts_store.pop((qc, kt))
            pts = (pt0, pt0, pt1, pt1)
            # AV with ones column: M=33, out rows 0:33 / 64:97 per bank
            for h in range(HEADS):
                psl = slice(512 * (h % 2), 512 * (h % 2 + 1))
                half = h % 2
                nc.tensor.matmul(
                    avbs[qc][h // 2][64 * half:64 * half + 33, :],
                    vT_bf[:, kt, h, 0:33],
                    pts[h][:, psl],
                    start=(kt == 0), stop=(kt == NK - 1),
                    skip_group_check=True, tile_position=(0, 64 * half))

        def emit_epilogue(qc):
            qsl = slice(512 * qc, 512 * (qc + 1))
            avb = avbs[qc]
            recs = []
            for b in range(2):
                rec_f = small.tile([128, 512], F32, tag="rec_f", name=f"rec{qc}_{b}")
                nc.vector.reciprocal_approx_fast(rec_f[0:97, :], avb[b][0:97, :])
                rec_bf = small.tile([128, 512], BF16, tag="rec_bf", name=f"recb{qc}_{b}")
                nc.gpsimd.tensor_copy(rec_bf[32:33, :], rec_f[32:33, :])
                nc.gpsimd.tensor_copy(rec_bf[96:97, :], rec_f[96:97, :])
                recs.append(rec_bf)
            bct = stp.tile([128, 1024], F32, tag="st", name=f"bc{qc}")
            hids = []
            for b in range(2):
                bc = bct[:, 512 * b:512 * (b + 1)]
                for half in range(2):
                    r = 64 * half + 32
                    nc.tensor.matmul(bc[64 * half:64 * half + 32, :],
                                     ones_bf[r:r + 1, 0:32], recs[b][r:r + 1, :],
                                     start=True, stop=True, skip_group_check=True,
                                     tile_position=(r - r % 32, 64 * half))
                bc_sb = small.tile([128, 512], F32, tag="bc_sb", name=f"bcs{qc}_{b}")
                nc.vector.tensor_copy(bc_sb[0:97, :], bc[0:97, :])
                hid = small.tile([128, 512], BF16, tag="hid", name=f"hid{qc}_{b}")
                # mul over rows 0:97 in one op (rows 33:63 get garbage, fixed
                # by the memsets below before the y matmul reads hid)
                nc.vector.tensor_mul(hid[0:97, :], avb[b][0:97, :],
                                     bc_sb[0:97, :])
                nc.gpsimd.memset(hid[32:64, :], 0.0)
                nc.gpsimd.memset(hid[96:128, :], 0.0)
                if b == 0:
                    # ones row 32 of hids[0] picks up the bias row of woA
                    nc.gpsimd.memset(hid[32:33, :], 1.0)
                hids.append(hid)

            yt = stp.tile([128, 1024], F32, tag="st", name=f"y{qc}")
            for oc in range(2):
                yps = yt[:, 512 * oc:512 * (oc + 1)]
                nc.tensor.matmul(yps, woA_bf[:, 128 * oc:128 * (oc + 1)],
                                 hids[0][:], start=True, stop=False,
                                 skip_group_check=True)
                nc.tensor.matmul(yps, woB_bf[:, 128 * oc:128 * (oc + 1)],
                                 hids[1][:], start=False, stop=True,
                                 skip_group_check=True)
                ysb = small.tile([128, 512], F32, tag="ysb", name=f"ysb{qc}_{oc}")
                nc.vector.tensor_copy(ysb[:], yps)
                dma_engines[oc % 2].dma_start(out[128 * oc:128 * (oc + 1), qsl], ysb[:])

        # gemm batches trickled ahead of need
        pre_gemms = {kt: [] for kt in range(NK)}
        for j in range(2, 8):
            pre_gemms[4 * j - 8].append(("k", j))
        for t in range(1, 8):
            pre_gemms[4 * t - 3].append(("v", t))

        def run_pre_gemms(kt):
            for kind, idx in pre_gemms[kt]:
                if kind == "k":
                    emit_k_gemm(idx)
                else:
                    emit_vT_batch(idx)

        emit_k_gemm(0)
        emit_k_gemm(1)
        emit_vT_batch(0)

        # software pipeline: AV lags ST/exp by one step; qc1's first PIPE
        # AVs are deferred past qc0's epilogue (they reuse its PSUM banks).
        PIPE = 6
        emit_st_exp(0, 0)
        for kt in range(1, NK):
            run_pre_gemms(kt - 1)
            emit_st_exp(0, kt)
            emit_av(0, kt - 1)
        run_pre_gemms(NK - 1)
        emit_st_exp(1, 0)
        emit_av(0, NK - 1)
        for kt in range(1, PIPE + 1):
            emit_st_exp(1, kt)
        emit_epilogue(0)
        for j in range(PIPE):
            emit_av(1, j)
        for kt in range(PIPE + 1, NK):
            emit_st_exp(1, kt)
            emit_av(1, kt - 1)
        emit_av(1, NK - 1)
        emit_epilogue(1)
    return nc


_NC_CACHE = None


def _get_nc():
    global _NC_CACHE
    if _NC_CACHE is None:
        nc = build_nc()
        nc.compile()
        _NC_CACHE = nc
    return _NC_CACHE


def _prep_weights(w_qkv, w_out, b_out):
    # w_qkv rows are interleaved: row (h*32+d)*3 + {0:q, 1:k, 2:v}
    w = np.asarray(w_qkv, np.float32).reshape(HEADS, DH, 3, C)
    import ml_dtypes

    def to_pcc(m):   # [C, 128] -> [p, cc, 128] bf16
        return np.ascontiguousarray(
            m.reshape(2, 128, 128).transpose(1, 0, 2)).astype(ml_dtypes.bfloat16)
    wq = to_pcc(w[:, :, 0, :].reshape(128, C).T)
    wk = to_pcc(w[:, :, 1, :].reshape(128, C).T)
    wv = to_pcc(w[:, :, 2, :].reshape(128, C).T)
    woT = np.asarray(w_out, np.float32).T                        # [128 c', C]
    woA = np.zeros((128, C), np.float32)
    woB = np.zeros((128, C), np.float32)
    woA[0:32] = woT[0:32]       # head 0
    woA[64:96] = woT[32:64]     # head 1
    woB[0:32] = woT[64:96]      # head 2
    woB[64:96] = woT[96:128]    # head 3
    woA[32] = np.asarray(b_out, np.float32)   # bias row (hid row 32 = ones)
    woA = woA.astype(ml_dtypes.bfloat16)
    woB = woB.astype(ml_dtypes.bfloat16)
    return wq, wk, wv, woA, woB


def kernel(x, w_qkv, w_out, b_out):
    import ml_dtypes
    x = np.asarray(x, np.float32)
    b, c, h, w = x.shape
    hw = h * w
    xf = np.ascontiguousarray(x.reshape(b, c, hw)).astype(ml_dtypes.bfloat16)
    wq, wk, wv, woA, woB = _prep_weights(w_qkv, w_out, b_out)

    in_maps = []
    for core in range(8):
        bi, qi = core // 4, core % 4
        # permute key pieces so piece 0 is this core's query chunk
        order = [qi] + [j for j in range(4) if j != qi]
        xb = np.ascontiguousarray(
            np.concatenate([xf[bi][:, 1024 * j:1024 * (j + 1)] for j in order],
                           axis=1))
        in_maps.append({
            "x": xb,
            "wq": wq, "wk": wk, "wv": wv, "woA": woA, "woB": woB,
        })

    nc = _get_nc()
    res = run_bass_kernel_spmd(nc, in_maps, core_ids=list(range(8)))
    y = np.empty((b, c, hw), np.float32)
    for core in range(8):
        bi, qi = core // 4, core % 4
        y[bi, :, QC * qi:QC * (qi + 1)] = res.results[core]["out"]
    return y.reshape(b, c, h, w)


# revision 27
# speedup vs baseline: 1.0138x; 1.0138x over previous
"""Trainium2 Bass kernel for nn_Attention_36601711297049.

Self-attention (4 heads, dim_head 32) over N=4096 tokens, batch 2:
  qkv = w_qkv @ x ; sim = scale * q^T k ; attn = softmax(sim) ;
  out = attn @ v ; y = w_out @ out + b_out

Sharding: 8 cores = 2 batches x 4 query-chunks (1024 queries each).
Each core computes k, v for the full batch plus q for its own chunk, runs
flash-style attention in S^T layout ([keys, queries] so AV needs no
transposes), and applies the output projection locally. No collectives.
Softmax skips max-subtraction (logits are ~N(0,1), safely in fp32 range).

Host-side trick: each core's x arrives with key pieces permuted so piece 0
IS the core's query chunk (softmax/AV are key-order invariant) - the q gemm
starts as soon as the first DMA lands.

ScalarE exp is the floor (~2.4us per iteration); the PE work (S^T 4-tile
blocks, AV col-pairs lagging one step, batched qkv gemms) hides under it.
PSUM pool tiles rotate as separate memrefs - a single shared tile would
serialize on the framework's coarse write-after-read tracking.
"""
import sys

for p in ("/opt/trn_rl_repo", "/root/.axon_site/_ro/trn_rl_repo"):
    if p not in sys.path:
        sys.path.insert(0, p)

import numpy as np
from contextlib import ExitStack

import concourse.bass as bass
from concourse import bacc
import concourse.tile as tile
from concourse import mybir
from concourse.bass_utils import run_bass_kernel_spmd

F32 = mybir.dt.float32
BF16 = mybir.dt.bfloat16
AF = mybir.ActivationFunctionType

HEADS = 4
DH = 32
C = 256          # channels
N = 4096         # h*w tokens per batch
QC = 1024        # queries per core
NK = N // 128    # 128-key tiles
SCALE = float(DH) ** -0.5


def build_nc():
    nc = bacc.Bacc("TRN2", target_bir_lowering=False)
    x = nc.dram_tensor("x", [C, N], BF16, kind="ExternalInput")
    wq = nc.dram_tensor("wq", [128, 2, 128], BF16, kind="ExternalInput")  # [p, cc, (h,d)]
    wk = nc.dram_tensor("wk", [128, 2, 128], BF16, kind="ExternalInput")
    wv = nc.dram_tensor("wv", [128, 2, 128], BF16, kind="ExternalInput")
    woA = nc.dram_tensor("woA", [128, C], BF16, kind="ExternalInput")  # w_out^T h0/h1 + bias row
    woB = nc.dram_tensor("woB", [128, C], BF16, kind="ExternalInput")  # w_out^T h2/h3, zero-padded
    out = nc.dram_tensor("out", [C, QC], F32, kind="ExternalOutput")

    with tile.TileContext(nc) as tc, ExitStack() as ctx:
        big = ctx.enter_context(tc.tile_pool(name="big", bufs=1))
        small = ctx.enter_context(tc.tile_pool(name="small", bufs=2))
        ptp = ctx.enter_context(tc.tile_pool(name="ptp", bufs=18))
        stp = ctx.enter_context(tc.tile_pool(name="stp", bufs=3, space="PSUM"))
        avp = ctx.enter_context(tc.tile_pool(name="avp", bufs=2, space="PSUM"))

        # warm the exp table set early (one tiny ACT forces the table load)
        dummy = small.tile([1, 8], F32, tag="dummy")
        nc.vector.memset(dummy[:], 0.0)
        nc.scalar.activation(dummy[:], dummy[:], AF.Exp)

        # ---- weights first (tiny, on sync), then x pieces ----
        wq_bf = big.tile([128, 2, 128], BF16, tag="wq_bf")
        wk_bf = big.tile([128, 2, 128], BF16, tag="wk_bf")
        wv_bf = big.tile([128, 2, 128], BF16, tag="wv_bf")
        for (dram, sbuf) in ((wq, wq_bf), (wk, wk_bf), (wv, wv_bf)):
            nc.sync.dma_start(sbuf[:], dram[:])
        woA_bf = big.tile([128, 256], BF16, tag="woA_bf")
        woB_bf = big.tile([128, 256], BF16, tag="woB_bf")
        nc.sync.dma_start(woA_bf[:], woA[:])
        nc.sync.dma_start(woB_bf[:], woB[:])
        ones_bf = big.tile([128, DH], BF16, tag="ones_bf")
        nc.vector.memset(ones_bf[:], 1.0)

        x_bf = big.tile([128, 2, N], BF16, tag="x_bf")
        dma_engines = (nc.sync, nc.gpsimd)

        # piece 0 = this core's query chunk; first halves land first so the
        # q(0)/k(0) gemms start early. scalar queue helps only here.
        nc.gpsimd.dma_start(x_bf[:, 0, 0:512], x[0:128, 0:512])
        nc.scalar.dma_start(x_bf[:, 1, 0:512], x[128:256, 0:512])
        nc.sync.dma_start(x_bf[:, 0, 512:1024], x[0:128, 512:1024])
        nc.gpsimd.dma_start(x_bf[:, 1, 512:1024], x[128:256, 512:1024])

        def dma_x_piece(piece, di):
            sl = slice(1024 * piece, 1024 * (piece + 1))
            for cc in range(2):
                dma_engines[(di + cc) % 2].dma_start(
                    x_bf[:, cc, sl], x[128 * cc:128 * (cc + 1), sl])

        for piece in range(1, 4):
            dma_x_piece(piece, 2 * piece)

        # ---- q = wq^T x[:, :, 0:QC] : [128 (h,d), QC] bf16 ----
        q_bf = big.tile([128, QC], BF16, tag="q_bf")

        def emit_q_gemm(nch):
            ps = stp.tile([128, 1024], F32, tag="st", name=f"q_ps{nch}")
            for cc in range(2):
                nc.tensor.matmul(ps[:, :512], wq_bf[:, cc, :],
                                 x_bf[:, cc, 512 * nch:512 * (nch + 1)],
                                 start=(cc == 0), stop=(cc == 1),
                                 skip_group_check=True)
            nc.vector.tensor_copy(q_bf[:, 512 * nch:512 * (nch + 1)], ps[:, :512])

        # ---- k = wk^T x and vT = x^T wv, batched, trickled into the loop ----
        k_bf = big.tile([128, N], BF16, tag="k_bf")
        vT_bf = big.tile([128, NK, 4, 34], BF16, tag="vT_bf")
        for h in range(HEADS):
            nc.vector.memset(vT_bf[:, :, h, 32:33], 1.0)

        def emit_k_gemm(nch):  # 512 keys
            ps = stp.tile([128, 1024], F32, tag="st", name=f"k_ps{nch}")
            for cc in range(2):
                nc.tensor.matmul(ps[:, :512], wk_bf[:, cc, :],
                                 x_bf[:, cc, 512 * nch:512 * (nch + 1)],
                                 start=(cc == 0), stop=(cc == 1),
                                 skip_group_check=True)
            nc.vector.tensor_copy(k_bf[:, 512 * nch:512 * (nch + 1)], ps[:, :512])

        def emit_vT_batch(t):  # 4 key tiles (kt = 4t..4t+3) back-to-back
            ps = stp.tile([128, 1024], F32, tag="st", name=f"v_ps{t}")
            for j in range(4):
                kt = 4 * t + j
                for cc in range(2):
                    nc.tensor.matmul(ps[:, 128 * j:128 * (j + 1)],
                                     x_bf[:, cc, 128 * kt:128 * (kt + 1)],
                                     wv_bf[:, cc, :],
                                     start=(cc == 0), stop=(cc == 1),
                                     skip_group_check=True)
            for j in range(4):
                kt = 4 * t + j
                nc.vector.tensor_copy(
                    vT_bf[:, kt, :, 0:32],
                    ps[:, 128 * j:128 * (j + 1)].rearrange(
                        "p (h d) -> p h d", d=32))

        # ---- attention main loop ----
        avbs = {}
        pts_store = {}

        def emit_st_exp(qc, kt):
            """S^T 4-tile block + 2 exps for (qc, kt)."""
            qsl = slice(512 * qc, 512 * (qc + 1))
            st0 = stp.tile([128, 1024], F32, tag="st", name=f"st0_{qc}_{kt}")
            st1 = stp.tile([128, 1024], F32, tag="st", name=f"st1_{qc}_{kt}")
            sts = (st0, st0, st1, st1)
            for h in range(HEADS):
                nc.tensor.matmul(
                    sts[h][:, 512 * (h % 2):512 * (h % 2 + 1)],
                    k_bf[32 * h:32 * (h + 1), 128 * kt:128 * (kt + 1)],
                    q_bf[32 * h:32 * (h + 1), qsl],
                    start=True, stop=True, skip_group_check=True,
                    tile_position=(32 * h, 0))
            pt0 = ptp.tile([128, 1024], BF16, tag="pt", name=f"pt0_{qc}_{kt}")
            pt1 = ptp.tile([128, 1024], BF16, tag="pt", name=f"pt1_{qc}_{kt}")
            nc.scalar.activation(pt0[:], st0[:], AF.Exp, scale=SCALE)
            nc.scalar.activation(pt1[:], st1[:], AF.Exp, scale=SCALE)
            pts_store[(qc, kt)] = (pt0, pt1)

        def emit_av(qc, kt):
            if kt == 0:
                avbs[qc] = [avp.tile([128, 512], F32, tag="acc", name=f"av{qc}_{b}")
                            for b in range(2)]
            pt0, pt1 = pts_store.pop((qc, kt))
            pts = (pt0, pt0, pt1, pt1)
            # AV with ones column: M=33, out rows 0:33 / 64:97 per bank
            for h in range(HEADS):
                psl = slice(512 * (h % 2), 512 * (h % 2 + 1))
                half = h % 2
                nc.tensor.matmul(
                    avbs[qc][h // 2][64 * half:64 * half + 33, :],
                    vT_bf[:, kt, h, 0:33],
                    pts[h][:, psl],
                    start=(kt == 0), stop=(kt == NK - 1),
                    skip_group_check=True, tile_position=(0, 64 * half))

        def emit_epilogue(qc):
            qsl = slice(512 * qc, 512 * (qc + 1))
            avb = avbs[qc]
            recs = []
            for b in range(2):
                rec_f = small.tile([128, 512], F32, tag="rec_f", name=f"rec{qc}_{b}")
                nc.vector.reciprocal_approx_fast(rec_f[0:97, :], avb[b][0:97, :])
                rec_bf = small.tile([128, 512], BF16, tag="rec_bf", name=f"recb{qc}_{b}")
                nc.gpsimd.tensor_copy(rec_bf[32:33, :], rec_f[32:33, :])
                nc.gpsimd.tensor_copy(rec_bf[96:97, :], rec_f[96:97, :])
                recs.append(rec_bf)
            # one PSUM tile for the whole epilogue: bc in cols 0:1024 first,
            # then (after the muls consumed bc) y reuses the same columns
            ept = stp.tile([128, 1024], F32, tag="st", name=f"ep{qc}")
            hids = []
            for b in range(2):
                bc = ept[:, 512 * b:512 * (b + 1)]
                for half in range(2):
                    r = 64 * half + 32
                    nc.tensor.matmul(bc[64 * half:64 * half + 32, :],
                                     ones_bf[r:r + 1, 0:32], recs[b][r:r + 1, :],
                                     start=True, stop=True, skip_group_check=True,
                                     tile_position=(r - r % 32, 64 * half))
                bc_sb = small.tile([128, 512], F32, tag="bc_sb", name=f"bcs{qc}_{b}")
                nc.vector.tensor_copy(bc_sb[0:97, :], bc[0:97, :])
                hid = small.tile([128, 512], BF16, tag="hid", name=f"hid{qc}_{b}")
                # mul over rows 0:97 in one op (rows 33:63 get garbage, fixed
                # by the memsets below before the y matmul reads hid)
                nc.vector.tensor_mul(hid[0:97, :], avb[b][0:97, :],
                                     bc_sb[0:97, :])
                nc.gpsimd.memset(hid[32:64, :], 0.0)
                nc.gpsimd.memset(hid[96:128, :], 0.0)
                if b == 0:
                    # ones row 32 of hids[0] picks up the bias row of woA
                    nc.gpsimd.memset(hid[32:33, :], 1.0)
                hids.append(hid)

            for oc in range(2):
                yps = ept[:, 512 * oc:512 * (oc + 1)]
                nc.tensor.matmul(yps, woA_bf[:, 128 * oc:128 * (oc + 1)],
                                 hids[0][:], start=True, stop=False,
                                 skip_group_check=True)
                nc.tensor.matmul(yps, woB_bf[:, 128 * oc:128 * (oc + 1)],
                                 hids[1][:], start=False, stop=True,
                                 skip_group_check=True)
                ysb = small.tile([128, 512], F32, tag="ysb", name=f"ysb{qc}_{oc}")
                nc.vector.tensor_copy(ysb[:], yps)
                dma_engines[oc % 2].dma_start(out[128 * oc:128 * (oc + 1), qsl], ysb[:])

        # gemm batches trickled ahead of need
        pre_gemms = {kt: [] for kt in range(NK)}
        for j in range(2, 8):
            pre_gemms[4 * j - 8].append(("k", j))
        for t in range(1, 8):
            pre_gemms[4 * t - 3].append(("v", t))

        def run_pre_gemms(kt):
            for kind, idx in pre_gemms[kt]:
                if kind == "k":
                    emit_k_gemm(idx)
                else:
                    emit_vT_batch(idx)

        emit_q_gemm(0)
        emit_k_gemm(0)
        emit_q_gemm(1)
        emit_k_gemm(1)
        emit_vT_batch(0)

        # software pipeline: AV lags ST/exp by one step; qc1's first PIPE
        # AVs are deferred past qc0's epilogue (they reuse its PSUM banks).
        PIPE = 6
        emit_st_exp(0, 0)
        for kt in range(1, NK):
            run_pre_gemms(kt - 1)
            emit_st_exp(0, kt)
            emit_av(0, kt - 1)
        run_pre_gemms(NK - 1)
        emit_st_exp(1, 0)
        emit_av(0, NK - 1)
        for kt in range(1, PIPE + 1):
            emit_st_exp(1, kt)
        emit_epilogue(0)
        for j in range(PIPE):
            emit_av(1, j)
        for kt in range(PIPE + 1, NK):
            emit_st_exp(1, kt)
            emit_av(1, kt - 1)
        emit_av(1, NK - 1)
        emit_epilogue(1)
    return nc


_NC_CACHE = None


def _get_nc():
    global _NC_CACHE
    if _NC_CACHE is None:
        nc = build_nc()
        nc.compile()
        _NC_CACHE = nc
    return _NC_CACHE


def _prep_weights(w_qkv, w_out, b_out):
    # w_qkv rows are interleaved: row (h*32+d)*3 + {0:q, 1:k, 2:v}
    w = np.asarray(w_qkv, np.float32).reshape(HEADS, DH, 3, C)
    import ml_dtypes

    def to_pcc(m):   # [C, 128] -> [p, cc, 128] bf16
        return np.ascontiguousarray(
            m.reshape(2, 128, 128).transpose(1, 0, 2)).astype(ml_dtypes.bfloat16)
    wq = to_pcc(w[:, :, 0, :].reshape(128, C).T)
    wk = to_pcc(w[:, :, 1, :].reshape(128, C).T)
    wv = to_pcc(w[:, :, 2, :].reshape(128, C).T)
    woT = np.asarray(w_out, np.float32).T                        # [128 c', C]
    woA = np.zeros((128, C), np.float32)
    woB = np.zeros((128, C), np.float32)
    woA[0:32] = woT[0:32]       # head 0
    woA[64:96] = woT[32:64]     # head 1
    woB[0:32] = woT[64:96]      # head 2
    woB[64:96] = woT[96:128]    # head 3
    woA[32] = np.asarray(b_out, np.float32)   # bias row (hid row 32 = ones)
    woA = woA.astype(ml_dtypes.bfloat16)
    woB = woB.astype(ml_dtypes.bfloat16)
    return wq, wk, wv, woA, woB


def kernel(x, w_qkv, w_out, b_out):
    import ml_dtypes
    x = np.asarray(x, np.float32)
    b, c, h, w = x.shape
    hw = h * w
    xf = np.ascontiguousarray(x.reshape(b, c, hw)).astype(ml_dtypes.bfloat16)
    wq, wk, wv, woA, woB = _prep_weights(w_qkv, w_out, b_out)

    in_maps = []
    for core in range(8):
        bi, qi = core // 4, core % 4
        # permute key pieces so piece 0 is this core's query chunk
        order = [qi] + [j for j in range(4) if j != qi]
        xb = np.ascontiguousarray(
            np.concatenate([xf[bi][:, 1024 * j:1024 * (j + 1)] for j in order],
                           axis=1))
        in_maps.append({
            "x": xb,
            "wq": wq, "wk": wk, "wv": wv, "woA": woA, "woB": woB,
        })

    nc = _get_nc()
    res = run_bass_kernel_spmd(nc, in_maps, core_ids=list(range(8)))
    y = np.empty((b, c, hw), np.float32)
    for core in range(8):
        bi, qi = core // 4, core % 4
        y[bi, :, QC * qi:QC * (qi + 1)] = res.results[core]["out"]
    return y.reshape(b, c, h, w)


# revision 28
# speedup vs baseline: 1.0325x; 1.0185x over previous
"""Trainium2 Bass kernel for nn_Attention_36601711297049.

Self-attention (4 heads, dim_head 32) over N=4096 tokens, batch 2:
  qkv = w_qkv @ x ; sim = scale * q^T k ; attn = softmax(sim) ;
  out = attn @ v ; y = w_out @ out + b_out

Sharding: 8 cores = 2 batches x 4 query-chunks (1024 queries each).
Each core computes k, v for the full batch plus q for its own chunk, runs
flash-style attention in S^T layout ([keys, queries] so AV needs no
transposes), and applies the output projection locally. No collectives.
Softmax skips max-subtraction (logits are ~N(0,1), safely in fp32 range).

Host-side trick: each core's x arrives with key pieces permuted so piece 0
IS the core's query chunk (softmax/AV are key-order invariant) - the q gemm
starts as soon as the first DMA lands.

ScalarE exp is the floor (~2.4us per iteration); the PE work (S^T 4-tile
blocks, AV col-pairs lagging one step, batched qkv gemms) hides under it.
PSUM pool tiles rotate as separate memrefs - a single shared tile would
serialize on the framework's coarse write-after-read tracking.
"""
import sys

for p in ("/opt/trn_rl_repo", "/root/.axon_site/_ro/trn_rl_repo"):
    if p not in sys.path:
        sys.path.insert(0, p)

import numpy as np
from contextlib import ExitStack

import concourse.bass as bass
from concourse import bacc
import concourse.tile as tile
from concourse import mybir
from concourse.bass_utils import run_bass_kernel_spmd

F32 = mybir.dt.float32
BF16 = mybir.dt.bfloat16
AF = mybir.ActivationFunctionType

HEADS = 4
DH = 32
C = 256          # channels
N = 4096         # h*w tokens per batch
QC = 1024        # queries per core
NK = N // 128    # 128-key tiles
SCALE = float(DH) ** -0.5


def build_nc():
    nc = bacc.Bacc("TRN2", target_bir_lowering=False)
    x = nc.dram_tensor("x", [C, N], BF16, kind="ExternalInput")
    wq = nc.dram_tensor("wq", [128, 2, 128], BF16, kind="ExternalInput")  # [p, cc, (h,d)]
    wk = nc.dram_tensor("wk", [128, 2, 128], BF16, kind="ExternalInput")
    wv = nc.dram_tensor("wv", [128, 2, 128], BF16, kind="ExternalInput")
    woA = nc.dram_tensor("woA", [128, C], BF16, kind="ExternalInput")  # w_out^T h0/h1 + bias row
    woB = nc.dram_tensor("woB", [128, C], BF16, kind="ExternalInput")  # w_out^T h2/h3, zero-padded
    out = nc.dram_tensor("out", [C, QC], F32, kind="ExternalOutput")

    with tile.TileContext(nc) as tc, ExitStack() as ctx:
        big = ctx.enter_context(tc.tile_pool(name="big", bufs=1))
        small = ctx.enter_context(tc.tile_pool(name="small", bufs=2))
        ptp = ctx.enter_context(tc.tile_pool(name="ptp", bufs=26))
        stp = ctx.enter_context(tc.tile_pool(name="stp", bufs=3, space="PSUM"))
        avp = ctx.enter_context(tc.tile_pool(name="avp", bufs=2, space="PSUM"))

        # warm the exp table set early (one tiny ACT forces the table load)
        dummy = small.tile([1, 8], F32, tag="dummy")
        nc.vector.memset(dummy[:], 0.0)
        nc.scalar.activation(dummy[:], dummy[:], AF.Exp)

        # ---- weights first (tiny, on sync), then x pieces ----
        wq_bf = big.tile([128, 2, 128], BF16, tag="wq_bf")
        wk_bf = big.tile([128, 2, 128], BF16, tag="wk_bf")
        wv_bf = big.tile([128, 2, 128], BF16, tag="wv_bf")
        for (dram, sbuf) in ((wq, wq_bf), (wk, wk_bf), (wv, wv_bf)):
            nc.sync.dma_start(sbuf[:], dram[:])
        woA_bf = big.tile([128, 256], BF16, tag="woA_bf")
        woB_bf = big.tile([128, 256], BF16, tag="woB_bf")
        nc.sync.dma_start(woA_bf[:], woA[:])
        nc.sync.dma_start(woB_bf[:], woB[:])
        ones_bf = big.tile([128, DH], BF16, tag="ones_bf")
        nc.vector.memset(ones_bf[:], 1.0)

        x_bf = big.tile([128, 2, N], BF16, tag="x_bf")
        dma_engines = (nc.sync, nc.gpsimd)

        # piece 0 = this core's query chunk; scalar queue helps only here
        nc.gpsimd.dma_start(x_bf[:, 0, 0:512], x[0:128, 0:512])
        nc.scalar.dma_start(x_bf[:, 0, 512:1024], x[0:128, 512:1024])
        nc.sync.dma_start(x_bf[:, 1, 0:1024], x[128:256, 0:1024])

        def dma_x_piece(piece, di):
            sl = slice(1024 * piece, 1024 * (piece + 1))
            for cc in range(2):
                dma_engines[(di + cc) % 2].dma_start(
                    x_bf[:, cc, sl], x[128 * cc:128 * (cc + 1), sl])

        for piece in range(1, 4):
            dma_x_piece(piece, 2 * piece)

        # ---- q = wq^T x[:, :, 0:QC] : [128 (h,d), QC] bf16 ----
        q_bf = big.tile([128, QC], BF16, tag="q_bf")
        for nch in range(QC // 512):
            ps = stp.tile([128, 1024], F32, tag="st", name=f"q_ps{nch}")
            for cc in range(2):
                nc.tensor.matmul(ps[:, :512], wq_bf[:, cc, :],
                                 x_bf[:, cc, 512 * nch:512 * (nch + 1)],
                                 start=(cc == 0), stop=(cc == 1),
                                 skip_group_check=True)
            nc.vector.tensor_copy(q_bf[:, 512 * nch:512 * (nch + 1)], ps[:, :512])

        # ---- k = wk^T x and vT = x^T wv, batched, trickled into the loop ----
        k_bf = big.tile([128, N], BF16, tag="k_bf")
        vT_bf = big.tile([128, NK, 4, 34], BF16, tag="vT_bf")
        for h in range(HEADS):
            nc.vector.memset(vT_bf[:, :, h, 32:33], 1.0)

        def emit_k_gemm(nch):  # 512 keys
            ps = stp.tile([128, 1024], F32, tag="st", name=f"k_ps{nch}")
            for cc in range(2):
                nc.tensor.matmul(ps[:, :512], wk_bf[:, cc, :],
                                 x_bf[:, cc, 512 * nch:512 * (nch + 1)],
                                 start=(cc == 0), stop=(cc == 1),
                                 skip_group_check=True)
            nc.vector.tensor_copy(k_bf[:, 512 * nch:512 * (nch + 1)], ps[:, :512])

        def emit_vT_batch(t):  # 4 key tiles (kt = 4t..4t+3) back-to-back
            ps = stp.tile([128, 1024], F32, tag="st", name=f"v_ps{t}")
            for j in range(4):
                kt = 4 * t + j
                for cc in range(2):
                    nc.tensor.matmul(ps[:, 128 * j:128 * (j + 1)],
                                     x_bf[:, cc, 128 * kt:128 * (kt + 1)],
                                     wv_bf[:, cc, :],
                                     start=(cc == 0), stop=(cc == 1),
                                     skip_group_check=True)
            for j in range(4):
                kt = 4 * t + j
                nc.vector.tensor_copy(
                    vT_bf[:, kt, :, 0:32],
                    ps[:, 128 * j:128 * (j + 1)].rearrange(
                        "p (h d) -> p h d", d=32))

        # ---- attention main loop ----
        avbs = {}
        pts_store = {}

        def emit_st_exp(qc, kt):
            """S^T 4-tile block + 2 exps for (qc, kt)."""
            qsl = slice(512 * qc, 512 * (qc + 1))
            st0 = stp.tile([128, 1024], F32, tag="st", name=f"st0_{qc}_{kt}")
            st1 = stp.tile([128, 1024], F32, tag="st", name=f"st1_{qc}_{kt}")
            sts = (st0, st0, st1, st1)
            for h in range(HEADS):
                nc.tensor.matmul(
                    sts[h][:, 512 * (h % 2):512 * (h % 2 + 1)],
                    k_bf[32 * h:32 * (h + 1), 128 * kt:128 * (kt + 1)],
                    q_bf[32 * h:32 * (h + 1), qsl],
                    start=True, stop=True, skip_group_check=True,
                    tile_position=(32 * h, 0))
            pt0 = ptp.tile([128, 1024], BF16, tag="pt", name=f"pt0_{qc}_{kt}")
            pt1 = ptp.tile([128, 1024], BF16, tag="pt", name=f"pt1_{qc}_{kt}")
            nc.scalar.activation(pt0[:], st0[:], AF.Exp, scale=SCALE)
            nc.scalar.activation(pt1[:], st1[:], AF.Exp, scale=SCALE)
            pts_store[(qc, kt)] = (pt0, pt1)

        def emit_av(qc, kt):
            if kt == 0:
                avbs[qc] = [avp.tile([128, 512], F32, tag="acc", name=f"av{qc}_{b}")
                            for b in range(2)]
            pt0, pt1 = pts_store.pop((qc, kt))
            pts = (pt0, pt0, pt1, pt1)
            # AV with ones column: M=33, out rows 0:33 / 64:97 per bank
            for h in range(HEADS):
                psl = slice(512 * (h % 2), 512 * (h % 2 + 1))
                half = h % 2
                nc.tensor.matmul(
                    avbs[qc][h // 2][64 * half:64 * half + 33, :],
                    vT_bf[:, kt, h, 0:33],
                    pts[h][:, psl],
                    start=(kt == 0), stop=(kt == NK - 1),
                    skip_group_check=True, tile_position=(0, 64 * half))

        def emit_epilogue(qc):
            qsl = slice(512 * qc, 512 * (qc + 1))
            avb = avbs[qc]
            recs = []
            for b in range(2):
                rec_f = small.tile([128, 512], F32, tag="rec_f", name=f"rec{qc}_{b}")
                nc.vector.reciprocal_approx_fast(rec_f[0:97, :], avb[b][0:97, :])
                rec_bf = small.tile([128, 512], BF16, tag="rec_bf", name=f"recb{qc}_{b}")
                nc.vector.tensor_copy(rec_bf[32:33, :], rec_f[32:33, :])
                nc.vector.tensor_copy(rec_bf[96:97, :], rec_f[96:97, :])
                recs.append(rec_bf)
            bct = stp.tile([128, 1024], F32, tag="st", name=f"bc{qc}")
            hids = []
            for b in range(2):
                bc = bct[:, 512 * b:512 * (b + 1)]
                for half in range(2):
                    r = 64 * half + 32
                    nc.tensor.matmul(bc[64 * half:64 * half + 32, :],
                                     ones_bf[r:r + 1, 0:32], recs[b][r:r + 1, :],
                                     start=True, stop=True, skip_group_check=True,
                                     tile_position=(r - r % 32, 64 * half))
                bc_sb = small.tile([128, 512], F32, tag="bc_sb", name=f"bcs{qc}_{b}")
                nc.vector.tensor_copy(bc_sb[0:97, :], bc[0:97, :])
                hid = small.tile([128, 512], BF16, tag="hid", name=f"hid{qc}_{b}")
                nc.vector.memset(hid[32:64, :], 0.0)
                nc.vector.memset(hid[96:128, :], 0.0)
                if b == 0:
                    # ones row 32 of hids[0] picks up the bias row of woA
                    nc.vector.memset(hid[32:33, :], 1.0)
                for half in range(2):
                    rows = slice(64 * half, 64 * half + 32)
                    nc.vector.tensor_mul(hid[rows, :], avb[b][rows, :],
                                         bc_sb[rows, :])
                hids.append(hid)

            yt = stp.tile([128, 1024], F32, tag="st", name=f"y{qc}")
            for oc in range(2):
                yps = yt[:, 512 * oc:512 * (oc + 1)]
                nc.tensor.matmul(yps, woA_bf[:, 128 * oc:128 * (oc + 1)],
                                 hids[0][:], start=True, stop=False,
                                 skip_group_check=True)
                nc.tensor.matmul(yps, woB_bf[:, 128 * oc:128 * (oc + 1)],
                                 hids[1][:], start=False, stop=True,
                                 skip_group_check=True)
                ysb = small.tile([128, 512], F32, tag="ysb", name=f"ysb{qc}_{oc}")
                nc.vector.tensor_copy(ysb[:], yps)
                dma_engines[oc % 2].dma_start(out[128 * oc:128 * (oc + 1), qsl], ysb[:])

        # gemm batches trickled ahead of need
        pre_gemms = {kt: [] for kt in range(NK)}
        for j in range(2, 8):
            pre_gemms[4 * j - 8].append(("k", j))
        for t in range(1, 8):
            pre_gemms[4 * t - 3].append(("v", t))

        def run_pre_gemms(kt):
            for kind, idx in pre_gemms[kt]:
                if kind == "k":
                    emit_k_gemm(idx)
                else:
                    emit_vT_batch(idx)

        emit_k_gemm(0)
        emit_k_gemm(1)
        emit_vT_batch(0)

        # software pipeline: AV lags ST/exp by one step; qc1's first PIPE
        # AVs are deferred past qc0's epilogue (they reuse its PSUM banks).
        PIPE = 10
        emit_st_exp(0, 0)
        for kt in range(1, NK):
            run_pre_gemms(kt - 1)
            emit_st_exp(0, kt)
            emit_av(0, kt - 1)
        run_pre_gemms(NK - 1)
        emit_st_exp(1, 0)
        emit_av(0, NK - 1)
        for kt in range(1, PIPE + 1):
            emit_st_exp(1, kt)
        emit_epilogue(0)
        for j in range(PIPE):
            emit_av(1, j)
        for kt in range(PIPE + 1, NK):
            emit_st_exp(1, kt)
            emit_av(1, kt - 1)
        emit_av(1, NK - 1)
        emit_epilogue(1)
    return nc


_NC_CACHE = None


def _get_nc():
    global _NC_CACHE
    if _NC_CACHE is None:
        nc = build_nc()
        nc.compile()
        _NC_CACHE = nc
    return _NC_CACHE


def _prep_weights(w_qkv, w_out, b_out):
    # w_qkv rows are interleaved: row (h*32+d)*3 + {0:q, 1:k, 2:v}
    w = np.asarray(w_qkv, np.float32).reshape(HEADS, DH, 3, C)
    import ml_dtypes

    def to_pcc(m):   # [C, 128] -> [p, cc, 128] bf16
        return np.ascontiguousarray(
            m.reshape(2, 128, 128).transpose(1, 0, 2)).astype(ml_dtypes.bfloat16)
    wq = to_pcc(w[:, :, 0, :].reshape(128, C).T)
    wk = to_pcc(w[:, :, 1, :].reshape(128, C).T)
    wv = to_pcc(w[:, :, 2, :].reshape(128, C).T)
    woT = np.asarray(w_out, np.float32).T                        # [128 c', C]
    woA = np.zeros((128, C), np.float32)
    woB = np.zeros((128, C), np.float32)
    woA[0:32] = woT[0:32]       # head 0
    woA[64:96] = woT[32:64]     # head 1
    woB[0:32] = woT[64:96]      # head 2
    woB[64:96] = woT[96:128]    # head 3
    woA[32] = np.asarray(b_out, np.float32)   # bias row (hid row 32 = ones)
    woA = woA.astype(ml_dtypes.bfloat16)
    woB = woB.astype(ml_dtypes.bfloat16)
    return wq, wk, wv, woA, woB


def kernel(x, w_qkv, w_out, b_out):
    import ml_dtypes
    x = np.asarray(x, np.float32)
    b, c, h, w = x.shape
    hw = h * w
    xf = np.ascontiguousarray(x.reshape(b, c, hw)).astype(ml_dtypes.bfloat16)
    wq, wk, wv, woA, woB = _prep_weights(w_qkv, w_out, b_out)

    in_maps = []
    for core in range(8):
        bi, qi = core // 4, core % 4
        # permute key pieces so piece 0 is this core's query chunk
        order = [qi] + [j for j in range(4) if j != qi]
        xb = np.ascontiguousarray(
            np.concatenate([xf[bi][:, 1024 * j:1024 * (j + 1)] for j in order],
                           axis=1))
        in_maps.append({
            "x": xb,
            "wq": wq, "wk": wk, "wv": wv, "woA": woA, "woB": woB,
        })

    nc = _get_nc()
    res = run_bass_kernel_spmd(nc, in_maps, core_ids=list(range(8)))
    y = np.empty((b, c, hw), np.float32)
    for core in range(8):
        bi, qi = core // 4, core % 4
        y[bi, :, QC * qi:QC * (qi + 1)] = res.results[core]["out"]
    return y.reshape(b, c, h, w)
